# revision 4
# baseline (speedup 1.0000x reference)
"""Trainium2 Bass kernel for BrainFunctionalConnectivityFeatureExtractionModule.

Math (per batch b, all f32):
    w    = relu(adj + adj_bias)                       (16,16)
    d    = 1/sqrt(sum(w, axis=1) + 1e-5)              (16,)
    lap  = I - d[:,None] * w * d[None,:]              (16,16)
    t1   = lap @ x[b]                                 (16,256)
    cp   = interleave(ones, t1)                       (16,512)
    h    = relu(brelu_bias + cp @ cheb_w)             (16,64)
    out  = h @ fc_w.T + fc_b                          (16,387)

Since the even interleaved lanes of cp are all-ones,
    cp @ cheb_w = t1 @ cheb_w[1::2] + sum(cheb_w[0::2], axis=0)
and the lap-mix commutes with the W1 contraction, so per 512-row tile:
    y   = x @ W1                      W1 = cheb_w[1::2]   (512,64)
    h   = relu((I32 (x) lap) y + bias_h)                  (512,64)
    out = h @ fc_w.T + fc_b                               (512,387)

Device mapping: pure data parallel over 8 cores, B=8192 -> 1024 batches/core,
ROWS = 16384 rows/core in 32 macro tiles of 512 rows (= 32 16-node graphs).

The PE cost on trn2 is ~1 ns per MOVING column (output free size); stationary
(lhsT) loads are free.  Contracting with W1 FIRST makes every later stage
narrow (64 wide instead of 256):
  mmA  (8 mm): lhsT = xT chunk [c128, row128], rhs = W1 chunk [c128, 64]
               -> y[row128, 64] accum over 2 c-chunks       512 cols
  mix  (4 mm): lhsT = y chunk [row128, h64], rhs = I8 (x) lapT [128,128]
               -> hT[h64, row'128] per 128-row chunk        512 cols
  fc   (4 mm): lhsT = hT slice [65, row128] (ones row adds fc_b),
               rhs = fc_wT [65, 388]  -> out[row128, 388]  1552 cols
Total 2576 cols/tile (vs 3600 for the mix-first ordering).  The emission is
software-pipelined (mmA(i), mix(i-1), fc(i-2)) so the PE stream never waits
on the DVE/Act PSUM->SBUF copies between stages and stays out of the slow
post-idle p-state.

HBM traffic is halved vs f32 I/O (target_regime=memory): x is cast AND
pre-transposed on the host into [t][c][kc][row] (one contiguous 2 KiB line
per partition per tile), the output is written bf16 in PE-natural row order
(contiguous 3096 B lines) and re-ordered/upcast on the host.  End-to-end
rel-l2 error vs the f32 reference: 3.6e-3 (measured on HW and in numpy sim).
All matmul inputs bf16 (fp32 matmuls hit the 2-4x slower PE datapath).
"""

import numpy as np
from contextlib import ExitStack

B, E, C, H, OUT = 8192, 16, 256, 64, 387
NCORES = 8
ROWS = (B // NCORES) * E        # 16384 rows per core
NQ = 4                          # 128-row chunks per macro tile
TR = 128 * NQ                   # 512 macro-tile rows
NT = ROWS // TR                 # 32 macro tiles per core
KC = C // 128                   # 2 contraction chunks of 128
OUTP = OUT + 1                  # fc matmul N padded even

_cache = {}


def _build_module(nt=NT):
    import concourse.tile as tile
    from concourse import bacc, mybir

    f32 = mybir.dt.float32
    bf16 = mybir.dt.bfloat16
    Relu = mybir.ActivationFunctionType.Relu

    nc = bacc.Bacc("TRN2", target_bir_lowering=False, debug=False,
                   num_devices=NCORES)

    x_d = nc.dram_tensor("x", (nt * 128, KC * TR), bf16, kind="ExternalInput").ap()
    r_d = nc.dram_tensor("r", (128, 128), bf16, kind="ExternalInput").ap()
    w1_d = nc.dram_tensor("w1", (KC, 128, H), bf16, kind="ExternalInput").ap()
    bh_d = nc.dram_tensor("bh", (H, 1), f32, kind="ExternalInput").ap()
    fcw_d = nc.dram_tensor("fcw", (H + 1, OUTP), bf16, kind="ExternalInput").ap()
    o_d = nc.dram_tensor("o", (nt * TR, OUT), bf16, kind="ExternalOutput").ap()

    with tile.TileContext(nc) as tc:
        with ExitStack() as ctx:
            consts = ctx.enter_context(tc.tile_pool(name="consts", bufs=1))
            xp = ctx.enter_context(tc.tile_pool(name="xp", bufs=3))
            yp = ctx.enter_context(tc.tile_pool(name="yp", bufs=3))
            hp = ctx.enter_context(tc.tile_pool(name="hp", bufs=3))
            op = ctx.enter_context(tc.tile_pool(name="op", bufs=3))
            ypp = ctx.enter_context(tc.tile_pool(name="ypp", bufs=2, space="PSUM"))
            hpp = ctx.enter_context(tc.tile_pool(name="hpp", bufs=2, space="PSUM"))
            opp = ctx.enter_context(tc.tile_pool(name="opp", bufs=4, space="PSUM"))

            r_sb = consts.tile([128, 128], bf16)
            nc.sync.dma_start(r_sb, r_d)
            w1_sb = consts.tile([128, KC, H], bf16)
            nc.sync.dma_start(w1_sb, w1_d.rearrange("k p h -> p k h"))
            bh_sb = consts.tile([H, 1], f32)
            nc.sync.dma_start(bh_sb, bh_d)
            fcw_sb = consts.tile([H + 1, OUTP], bf16)
            nc.sync.dma_start(fcw_sb, fcw_d)

            # x: host pre-transposed; tile t, partition c holds rows 0..511
            # of c-chunks as one contiguous (kc row) line
            xv = x_d.rearrange("(t p) kr -> t p kr", p=128)
            # out: PE-natural order [t][p][q][o]; host re-orders rows
            ov = o_d.rearrange("(t p q) o -> t p (q o)", p=128, q=NQ)

            x_sb = [None] * nt
            y_ps = [None] * nt
            y_sb = [None] * nt
            hT_ps = [None] * nt
            hT_sb = [None] * nt

            def dma_in(i):
                x_sb[i] = xp.tile([128, KC * TR], bf16, name="xt")
                nc.sync.dma_start(x_sb[i], xv[i])

            def mm_a(i):
                # y[row, h] accumulated over the 2 c-chunks, per 128-row chunk
                y_ps[i] = ypp.tile([128, NQ, H], f32, name="yps")
                for q in range(NQ):
                    for k in range(KC):
                        nc.tensor.matmul(
                            y_ps[i][:, q, :],
                            lhsT=x_sb[i][:, k * TR + q * 128:k * TR + (q + 1) * 128],
                            rhs=w1_sb[:, k, :],
                            start=(k == 0),
                            stop=(k == KC - 1),
                        )
                x_sb[i] = None
                y_sb[i] = yp.tile([128, NQ, H], bf16, name="ysb")
                nc.vector.tensor_copy(y_sb[i], y_ps[i])
                y_ps[i] = None

            def mm_mix(i):
                # hT[h, row'] = y.T-mixed via I8 (x) lapT, per 128-row chunk
                hT_ps[i] = hpp.tile([H, TR], f32, name="hps")
                for q in range(NQ):
                    nc.tensor.matmul(
                        hT_ps[i][:, q * 128:(q + 1) * 128],
                        lhsT=y_sb[i][:, q, :],
                        rhs=r_sb,
                    )
                y_sb[i] = None
                hT_sb[i] = hp.tile([H + 1, TR], bf16, name="hsb")
                nc.gpsimd.memset(hT_sb[i][H:H + 1, :], 1.0)
                nc.scalar.activation(hT_sb[i][0:H, :], hT_ps[i], Relu, bias=bh_sb)
                hT_ps[i] = None

            def mm_fc(i):
                o_sb = op.tile([128, NQ * OUT], bf16)
                hT_v = hT_sb[i].rearrange("h (q n) -> h q n", q=NQ)
                for q in range(NQ):
                    o_ps = opp.tile([128, OUTP], f32)
                    nc.tensor.matmul(
                        o_ps,
                        lhsT=hT_v[:, q, :],
                        rhs=fcw_sb,
                    )
                    if q % 2 == 0:
                        nc.vector.tensor_copy(
                            o_sb[:, q * OUT:(q + 1) * OUT], o_ps[:, 0:OUT])
                    else:
                        nc.scalar.copy(
                            o_sb[:, q * OUT:(q + 1) * OUT], o_ps[:, 0:OUT])
                hT_sb[i] = None
                nc.sync.dma_start(ov[i], o_sb)

            dma_in(0)
            if nt > 1:
                dma_in(1)
            for i in range(nt):
                if i + 2 < nt:
                    dma_in(i + 2)
                mm_a(i)
                if i >= 1:
                    mm_mix(i - 1)
                if i >= 2:
                    mm_fc(i - 2)
            mm_mix(nt - 1)
            mm_fc(nt - 2)
            mm_fc(nt - 1)

    nc.finalize()
    return nc


def _host_prep(adj, adj_bias, cheb_w, brelu_bias, fc_w, fc_b):
    import ml_dtypes

    bf = ml_dtypes.bfloat16
    adj = np.asarray(adj, np.float32)
    w = np.maximum(adj + np.float32(adj_bias.reshape(())), 0.0)
    d = 1.0 / np.sqrt(w.sum(axis=1) + np.float32(1e-5))
    lap = np.eye(E, dtype=np.float32) - d[:, None] * w * d[None, :]

    # r = I_8 (x) lap^T : [p = b*16+j, n = b*16+i] -> lap[i, j]
    r = np.kron(np.eye(128 // E, dtype=np.float32), lap.T)

    cheb_w = np.asarray(cheb_w, np.float32)
    w1 = np.ascontiguousarray(cheb_w[1::2, :]).reshape(KC, 128, H)
    bias_h = (cheb_w[0::2, :].sum(axis=0)
              + np.asarray(brelu_bias, np.float32).reshape(H))
    fcw = np.zeros((H + 1, OUTP), np.float32)
    fcw[:H, :OUT] = np.asarray(fc_w, np.float32).T
    fcw[H, :OUT] = np.asarray(fc_b, np.float32)
    return {
        "r": r.astype(bf),
        "w1": np.ascontiguousarray(w1).astype(bf),
        "bh": bias_h.reshape(H, 1).astype(np.float32),
        "fcw": fcw.astype(bf),
    }


def _prep_x(x, nt=NT):
    """Cast to bf16 and pre-transpose into the stationary-operand layout:
    DRAM[t][c][kc][row] so each partition line is 2 KiB contiguous."""
    import ml_dtypes

    bf = ml_dtypes.bfloat16
    shards = np.asarray(x, np.float32).reshape(NCORES, ROWS, C)
    rows = nt * TR
    out = []
    for c in range(NCORES):
        a = shards[c, :rows].reshape(nt, TR, KC, 128)
        out.append(a.transpose(0, 3, 2, 1).astype(bf).reshape(nt * 128, KC * TR))
    return out


def _unshuffle_out(o, nt=NT):
    """Device writes [t][p][q][o] (row = t*512 + q*128 + p); restore natural
    row order and upcast to f32."""
    a = o.reshape(nt, 128, NQ, OUT).astype(np.float32)
    return a.transpose(0, 2, 1, 3).reshape(nt * TR // E, E, OUT)


def _run(inputs, trace=False, nt=NT, **kw):
    from concourse import bass_utils

    key = ("nc", nt)
    if key not in _cache:
        _cache[key] = _build_module(nt)
    nc = _cache[key]

    weights = _host_prep(inputs["adj"], inputs["adj_bias"], inputs["cheb_w"],
                         inputs["brelu_bias"], inputs["fc_w"], inputs["fc_b"])
    xs = _prep_x(inputs["x"], nt)

    in_maps = [dict(weights, x=xs[c]) for c in range(NCORES)]

    res = bass_utils.run_bass_kernel_spmd(
        nc, in_maps, core_ids=list(range(NCORES)), trace=trace, **kw)

    out = np.concatenate(
        [_unshuffle_out(res.results[c]["o"], nt) for c in range(NCORES)],
        axis=0)
    return out, res


def kernel(**inputs) -> np.ndarray:
    out, _ = _run(inputs, trace=False)
    return out
